# revision 2
# baseline (speedup 1.0000x reference)
"""CRF negative-log-likelihood loss kernel for Trainium2 (8 NeuronCores).

Problem: B=256, S=2048, T=64 CRF loss (torchcrf-style), mask all-ones.

Strategy
--------
Data-parallel over batch: each of the 8 cores gets 32 batch rows.

Denominator (log-partition): forward/backward meet-in-the-middle in the
exp domain.  The two chains are packed into one 128-partition stream
(block-diagonal lhsT: W for W^T @ E on top, W^T for W @ C below) so
each of the 1023 rounds is one PE matmul [128,128]@[128,32] plus one
DVE multiply [128,32] -- the DVE multiply (PSUM-read, 158 ns) is the
pacing cost.  A constant per-step prescale c0 keeps fp32 in range;
every RN rounds the state is renormalized by the bf16 reciprocal of
its per-direction mass.  The raw bf16 reciprocals (exactly as applied)
and the raw stitched partition value z are shipped to the host, which
reconstructs  den = ln(z) - sum ln(rmass) + S*c0.  No device-side Ln
-> the ACT engine keeps its Exp table loaded all kernel (no table
swaps).

Numerator: host folds start/end transitions AND the gold-path
transition rows (trans[tag_{s-1}, :]) into a separate numerator
emissions tensor em_n, so the numerator reduces to
sum_{b,s} em_n[b, s, tag[b,s]].  On device: per chunk, GpSimd
local_scatter builds the one-hot selection mask from host-prepared
int16 indices (2 instructions / chunk), then the PE contracts
one-hot^T @ em_n with two lanes packed per 128-wide stationary
(16 matmuls / chunk) accumulating into a single persistent PSUM tile;
the numerator is trace(accumulator), extracted once at the end.  The
DVE is left with only the recurrence.

Emissions travel as bf16 (half the DMA bytes); exp() output X stays
f32, stored j-major ([128, j, b]) so the per-round DVE read is
contiguous.

Per-core outputs: zraw[1,32] f32, rml[2, nren*32] bf16, gsum[1,2] f32.
Host: den_b = ln z_b - sum_r ln rml[:, r, b] + S*c0;
loss = (den_tot - num_tot) / B.
"""

import contextlib

import numpy as np
import ml_dtypes

F32_NP = np.float32
BF16_NP = ml_dtypes.bfloat16

B, S, T = 256, 2048, 64
NCORES = 8
BSH = B // NCORES  # 32
CHUNK = 128
C0 = 4.8204  # ~ ln(64 * e^0.5 * sinh(1)) : expected per-step log growth
RN = 64  # renorm every RN rounds

_NC_CACHE = {}


def build(n_chunks=16, bsh=BSH, nrep=1, fake_x=False, no_num=False,
          no_rounds=False, rn=RN, pround_bufs=4, spool_bufs=6,
          fake_x_dma=False, num_mode="pe2", act_k=0, x_bf16=False):
    """Build + compile the per-core Bass module. n_chunks*128 = seq len."""
    import concourse.bacc as bacc
    import concourse.mybir as mybir
    import concourse.tile as tile

    F32 = mybir.dt.float32
    BF16 = mybir.dt.bfloat16
    AF = mybir.ActivationFunctionType
    ALU = mybir.AluOpType

    s_len = n_chunks * CHUNK
    half = n_chunks // 2
    assert half * 2 == n_chunks and half >= 1
    n_rounds = half * CHUNK - 1
    nren = max((n_rounds - 1) // rn, 0) if not no_rounds else 0
    n_windows = half  # one window per 128 rounds

    nc = bacc.Bacc("TRN2", target_bir_lowering=False, debug=False,
                   num_devices=NCORES)

    em_x_d = nc.dram_tensor("emx", [half, 128, 128, bsh], BF16,
                            kind="ExternalInput")
    em_n_d = nc.dram_tensor("emn", [n_chunks, 128, bsh, T], BF16,
                            kind="ExternalInput")
    nidx_d = nc.dram_tensor("nidx", [n_chunks, 128, bsh], mybir.dt.int16,
                            kind="ExternalInput")
    trans_d = nc.dram_tensor("trans", [T, T], F32, kind="ExternalInput")
    bones_d = nc.dram_tensor("bones", [128, 2], BF16, kind="ExternalInput")
    bsel_d = nc.dram_tensor("bsel", [2, 128], BF16, kind="ExternalInput")
    ident_d = nc.dram_tensor("ident", [128, 128], F32, kind="ExternalInput")
    z_d = nc.dram_tensor("zraw", [1, bsh], F32, kind="ExternalOutput")
    rml_d = nc.dram_tensor("rml", [2, max(nren, 1) * bsh], BF16,
                           kind="ExternalOutput")
    gsum_d = nc.dram_tensor("gsum", [1, 2], F32, kind="ExternalOutput")

    with tile.TileContext(nc) as tc, nc.allow_low_precision(
            reason="bf16 state/weights validated against f64 reference"):
        with (
            tc.tile_pool(name="consts", bufs=1) as consts,
            tc.tile_pool(name="xchunk", bufs=3) as xpool,
            tc.tile_pool(name="xraw", bufs=3) as xrawpool,
            tc.tile_pool(name="emt", bufs=4) as empool,
            tc.tile_pool(name="oht", bufs=3) as ohpool,
            tc.tile_pool(name="state", bufs=spool_bufs) as spool,
            tc.tile_pool(name="small", bufs=4) as smallpool,
            tc.tile_pool(name="prod", bufs=3) as prodpool,
            tc.tile_pool(name="pround", bufs=pround_bufs,
                         space="PSUM") as pround,
            tc.tile_pool(name="pmisc", bufs=1, space="PSUM") as pmisc,
            tc.tile_pool(name="pnum", bufs=1, space="PSUM") as pnum,
        ):
            rep_ctx = (tc.For_i(0, nrep, 1) if nrep > 1
                       else contextlib.nullcontext())
            with rep_ctx:
                # ---------------- constants / setup ----------------
                ident = consts.tile([128, 128], F32, tag="ident")
                nc.sync.dma_start(ident[:], ident_d.ap())
                trans_sb = consts.tile([T, T], F32, tag="trans")
                nc.sync.dma_start(trans_sb[:], trans_d.ap())

                # block-diagonal lhsT (bf16): top-left W (for W^T @ E),
                # bottom-right W^T (for W @ C)
                blockw = consts.tile([128, 128], BF16, tag="blockw")
                nc.vector.memset(blockw[:], 0.0)
                nc.scalar.activation(blockw[0:T, 0:T], trans_sb[:], AF.Exp)
                tp = pmisc.tile([128, 128], F32, tag="m128")
                nc.tensor.matmul(tp[0:T, 0:T], trans_sb[:], ident[0:T, 0:T],
                                 start=True, stop=True)
                nc.scalar.activation(blockw[T:128, T:128], tp[0:T, 0:T],
                                     AF.Exp)

                blockones = consts.tile([128, 2], BF16, tag="blockones")
                nc.sync.dma_start(blockones[:], bones_d.ap())
                blocksel = consts.tile([2, 128], BF16, tag="blocksel")
                nc.sync.dma_start(blocksel[:], bsel_d.ap())
                ones64 = consts.tile([T, 1], F32, tag="ones64")
                nc.vector.memset(ones64[:], 1.0)
                ones128 = consts.tile([128, 1], F32, tag="ones128")
                nc.vector.memset(ones128[:], 1.0)
                ones128b = consts.tile([128, 1], BF16, tag="ones128b")
                nc.vector.memset(ones128b[:], 1.0)
                negc0 = consts.tile([128, 1], F32, tag="negc0")
                nc.vector.memset(negc0[:], -C0)

                # renorm log buffer: applied bf16 reciprocals
                rml = consts.tile([2, max(nren, 1) * bsh], BF16, tag="rml")
                if nren == 0:
                    nc.vector.memset(rml[:], 1.0)

                # scatter indices: nidx[p, g, i] = host-prepared int16 offsets
                if not no_num:
                    nidx = consts.tile([128, n_chunks, bsh], mybir.dt.int16,
                                       tag="nidx")
                    nc.sync.dma_start(
                        nidx[:], nidx_d.ap().rearrange("g p i -> p g i"))
                    ones_b = consts.tile([128, bsh], BF16, tag="onesb")
                    nc.vector.memset(ones_b[:], 1.0)

                # numerator accumulator
                nsum = 4  # PE column-sum slices per chunk
                wsum = (bsh * T) // nsum
                if not no_num:
                    if num_mode == "pe2":
                        num_ps = pnum.tile([128, 128], F32, tag="num_ps")
                        num_state = {"first": True, "last": None}
                    elif num_mode == "pool":
                        num_ps = pnum.tile([1, wsum], F32, tag="num_ps")
                        num_state = {"first": True, "last": None}
                    elif num_mode == "ttr":
                        acc = consts.tile([128, n_chunks], F32, tag="acc")
                        gscr = consts.tile([128, bsh, T], BF16, tag="gscr")
                    else:  # stt
                        nsub = 4
                        bsub = bsh // nsub
                        acc = consts.tile([128, n_chunks * nsub], F32,
                                          tag="acc")
                        gscr = consts.tile([128, bsub, T], BF16, tag="gscr")

                emg = {}      # em_n chunk g -> tile [128, bsh, T] bf16
                ohg = {}      # one-hot chunk g -> tile [128, bsh, T] bf16
                prg = {}      # product chunk g -> tile [128, bsh, T] bf16
                exraw = {}    # x-chunk c -> tile [128, 128, bsh] bf16

                def dma_x(c):
                    xr = xrawpool.tile([128, 128, bsh], BF16, tag="xr")
                    nc.sync.dma_start(xr[:], em_x_d.ap()[c])
                    exraw[c] = xr

                def dma_emn(g):
                    eg = empool.tile([128, bsh, T], BF16, tag="em")
                    nc.sync.dma_start(eg[:], em_n_d.ap()[g])
                    emg[g] = eg

                def num_quanta(g):
                    """Chunk-g numerator: Pool one-hot scatter + DVE accum."""
                    qs = []
                    if no_num:
                        return qs
                    hb = bsh // 2

                    def q_oh(g=g, h=0):
                        if h == 0:
                            oh = ohpool.tile([128, bsh, T], BF16, tag="oh")
                            ohg[g] = oh
                        nc.gpsimd.local_scatter(
                            ohg[g][:, h * hb:(h + 1) * hb, :],
                            ones_b[:, h * hb:(h + 1) * hb],
                            nidx[:, g, h * hb:(h + 1) * hb],
                            channels=128, num_elems=hb * T, num_idxs=hb)
                    qs.append(lambda g=g: q_oh(g, 0))
                    qs.append(lambda g=g: q_oh(g, 1))
                    if num_mode == "pe2":
                        npair = bsh // 2
                        for i in range(npair):
                            def q(g=g, i=i, last=(i == npair - 1)):
                                mm = nc.tensor.matmul(
                                    num_ps[:], ohg[g][:, 2 * i:2 * i + 2, :],
                                    emg[g][:, 2 * i:2 * i + 2, :],
                                    start=num_state["first"], stop=False,
                                    skip_group_check=True)
                                num_state["first"] = False
                                num_state["last"] = mm
                                if last:
                                    del emg[g]
                                    del ohg[g]
                            qs.append(q)
                    elif num_mode == "pool":
                        def q_mul(g=g, h=0):
                            if h == 0:
                                pr = prodpool.tile([128, bsh, T], BF16,
                                                   tag="pr")
                                prg[g] = pr
                            sl = slice(h * hb, (h + 1) * hb)
                            nc.gpsimd.tensor_mul(
                                prg[g][:, sl, :], emg[g][:, sl, :],
                                ohg[g][:, sl, :])
                            if h == 1:
                                del emg[g]
                                del ohg[g]
                        qs.append(lambda g=g: q_mul(g, 0))
                        qs.append(lambda g=g: q_mul(g, 1))
                        bsl = bsh // nsum
                        for si in range(nsum):
                            def q(g=g, si=si, last=(si == nsum - 1)):
                                mm = nc.tensor.matmul(
                                    num_ps[:], ones128b[:],
                                    prg[g][:, si * bsl:(si + 1) * bsl, :],
                                    start=num_state["first"], stop=False,
                                    skip_group_check=True)
                                num_state["first"] = False
                                num_state["last"] = mm
                                if last:
                                    del prg[g]
                            qs.append(q)
                    elif num_mode == "ttr":
                        def q_ttr(g=g):
                            nc.vector.tensor_tensor_reduce(
                                gscr[:], emg[g][:], ohg[g][:],
                                1.0, 0.0, op0=ALU.mult, op1=ALU.add,
                                accum_out=acc[:, g:g + 1])
                            del emg[g]
                            del ohg[g]
                        qs.append(q_ttr)
                    else:  # stt
                        for si in range(nsub):
                            def q(g=g, si=si, last=(si == nsub - 1)):
                                sl = slice(si * bsub, (si + 1) * bsub)
                                col = g * nsub + si
                                nc.vector.scalar_tensor_tensor(
                                    gscr[:], emg[g][:, sl, :], 1.0,
                                    ohg[g][:, sl, :],
                                    op0=ALU.bypass, op1=ALU.mult,
                                    accum_out=acc[:, col:col + 1])
                                if last:
                                    del emg[g]
                                    del ohg[g]
                            qs.append(q)
                    return qs

                XDT = BF16 if x_bf16 else F32

                def x_quanta(c):
                    """ACT-exp quanta producing X chunk c from em_x."""
                    xc = xpool.tile([128, 128, bsh], XDT, tag="xc")
                    if fake_x or fake_x_dma:
                        def q():
                            nc.vector.memset(xc[:], 0.0133)
                        return xc, [q]
                    qs = []
                    for hj in range(4):
                        def q(hj=hj):
                            sl = slice(hj * 32, (hj + 1) * 32)
                            nc.scalar.activation(
                                xc[:, sl, :], exraw[c][:, sl, :],
                                AF.Exp, bias=negc0[:])
                            if hj == 3:
                                del exraw[c]
                        qs.append(q)
                    return xc, qs

                # ---------------- main pipeline ----------------
                from collections import deque
                bg = deque()
                xchunks = {}
                if fake_x:
                    for c in (0, 1):
                        if c <= half - 1:
                            xc, qs = x_quanta(c)
                            [q() for q in qs]
                            xchunks[c] = xc
                else:
                    dma_x(0)
                    if half > 1:
                        dma_x(1)
                    if half > 2:
                        dma_x(2)
                    if not no_num:
                        dma_emn(0)
                        dma_emn(1)
                    xc, qs = x_quanta(0)
                    [q() for q in qs]
                    xchunks[0] = xc
                    if half > 1:
                        xc, qs = x_quanta(1)
                        [q() for q in qs]
                        xchunks[1] = xc

                state = spool.tile([128, bsh], BF16, tag="st")
                nc.vector.tensor_copy(state[:], xchunks[0][:, 0, :])

                ren_i = 0
                r_end = 0 if no_rounds else n_rounds
                for r in range(1, r_end + 1):
                    c, j = r >> 7, r & 127
                    if j == 1:
                        if not fake_x and c + 3 <= half - 1:
                            dma_x(c + 3)
                        if not fake_x and not no_num:
                            for g in (2 * (c + 1), 2 * (c + 1) + 1):
                                if g <= n_chunks - 1:
                                    dma_emn(g)
                        if c + 2 <= half - 1:
                            xc, qs = x_quanta(c + 2)
                            xchunks[c + 2] = xc
                            bg.extend(qs)
                            xchunks.pop(c - 1, None)
                        if not fake_x:
                            bg.extend(num_quanta(2 * c))
                            bg.extend(num_quanta(2 * c + 1))
                    if bg:
                        bg.popleft()()
                    p = pround.tile([128, bsh], F32, tag="p")
                    nc.tensor.matmul(p[:], blockw[:], state[:],
                                     start=True, stop=True)
                    state = spool.tile([128, bsh], BF16, tag="st")
                    if (r % 16) < act_k:
                        psb = smallpool.tile([128, bsh], BF16, tag="psb")
                        nc.scalar.activation(psb[:], p[:], AF.Copy)
                        nc.vector.tensor_mul(state[:], psb[:],
                                             xchunks[c][:, j, :])
                    else:
                        nc.vector.tensor_mul(state[:], p[:],
                                             xchunks[c][:, j, :])

                    if r % rn == 0 and r < n_rounds and ren_i < nren:
                        sl = slice(ren_i * bsh, (ren_i + 1) * bsh)
                        mass = pmisc.tile([2, bsh], F32, tag="m2")
                        nc.tensor.matmul(mass[:], blockones[:], state[:],
                                         start=True, stop=True)
                        nc.vector.reciprocal(rml[:, sl], mass[:])
                        rbc = pmisc.tile([128, 128], F32, tag="m128")
                        nc.tensor.matmul(rbc[:, 0:bsh], blocksel[:],
                                         rml[:, sl], start=True, stop=True)
                        nstate = spool.tile([128, bsh], BF16, tag="st")
                        nc.vector.tensor_mul(nstate[:], state[:],
                                             rbc[:, 0:bsh])
                        state = nstate
                        ren_i += 1

                while bg:
                    bg.popleft()()
                if no_rounds and not fake_x and not no_num:
                    for g in range(n_chunks):
                        if g >= 2:
                            dma_emn(g)
                        for q in num_quanta(g):
                            q()

                # ---------------- final combine ----------------
                # beta = W @ C on partitions 0..63 (aligned base-64 matmul)
                pf = pround.tile([128, bsh], F32, tag="p")
                nc.tensor.matmul(pf[0:T, :], blockw[T:128, T:128],
                                 state[T:128, :], start=True, stop=True)
                y = smallpool.tile([T, bsh], F32, tag="y")
                nc.vector.tensor_mul(y[:], state[0:T, :], pf[0:T, :])
                z = pmisc.tile([2, bsh], F32, tag="m2")
                nc.tensor.matmul(z[0:1, :], ones64[:], y[:],
                                 start=True, stop=True)
                z_sb = smallpool.tile([1, bsh], F32, tag="zsb")
                nc.vector.tensor_copy(z_sb[:], z[0:1, :])
                nc.sync.dma_start(z_d.ap(), z_sb[:])
                nc.sync.dma_start(rml_d.ap(), rml[:])

                # numerator finish
                gsb = smallpool.tile([1, 2], F32, tag="gsb")
                nc.vector.memset(gsb[:], 0.0)
                if not no_num:
                    if num_mode == "pe2":
                        if num_state["last"] is not None:
                            num_state["last"].ins.stop_tensor_calc = True
                            tr1 = smallpool.tile([128, 1], F32, tag="tr1")
                            trscr = smallpool.tile([128, 128], F32,
                                                   tag="trscr")
                            nc.vector.scalar_tensor_tensor(
                                trscr[:], ident[:], 1.0, num_ps[:],
                                op0=ALU.bypass, op1=ALU.mult,
                                accum_out=tr1[:])
                            gps = pmisc.tile([2, bsh], F32, tag="m2")
                            nc.tensor.matmul(gps[0:1, 0:1], ones128[:],
                                             tr1[:], start=True, stop=True)
                            nc.vector.tensor_copy(gsb[:, 0:1],
                                                  gps[0:1, 0:1])
                    elif num_mode == "pool":
                        if num_state["last"] is not None:
                            num_state["last"].ins.stop_tensor_calc = True
                            nc.vector.tensor_reduce(
                                gsb[:, 0:1], num_ps[:],
                                mybir.AxisListType.X, ALU.add)
                    else:
                        acc1 = smallpool.tile([128, 1], F32, tag="acc1")
                        nc.vector.tensor_reduce(
                            acc1[:], acc[:], mybir.AxisListType.X, ALU.add)
                        gps = pmisc.tile([2, bsh], F32, tag="m2")
                        nc.tensor.matmul(gps[0:1, 0:1], ones128[:], acc1[:],
                                         start=True, stop=True)
                        nc.vector.tensor_copy(gsb[:, 0:1], gps[0:1, 0:1])
                nc.sync.dma_start(gsum_d.ap(), gsb[:])

    nc.compile()
    return nc


def _get_nc(n_chunks=16, bsh=BSH):
    key = (n_chunks, bsh)
    if key not in _NC_CACHE:
        _NC_CACHE[key] = build(n_chunks, bsh)
    return _NC_CACHE[key]


def _consts():
    ident = np.eye(128, dtype=F32_NP)
    bones = np.zeros((128, 2), dtype=F32_NP)
    bones[0:T, 0] = 1.0
    bones[T:128, 1] = 1.0
    bsel = np.zeros((2, 128), dtype=F32_NP)
    bsel[0, 0:T] = 1.0
    bsel[1, T:128] = 1.0
    return ident, bones.astype(BF16_NP), bsel.astype(BF16_NP)


def make_in_maps(emissions, start_transitions, end_transitions, transitions,
                 tags, ncores=NCORES):
    """Host prep: fold start/end into em (denominator path), fold
    start/end + gold-path transition rows into em_n (numerator path),
    convert to bf16, build the DMA-friendly layouts, shard over cores."""
    em = np.asarray(emissions, dtype=F32_NP)
    start = np.asarray(start_transitions, dtype=F32_NP)
    end = np.asarray(end_transitions, dtype=F32_NP)
    trans = np.asarray(transitions, dtype=F32_NP).reshape(T, T)
    b_all, s_len = em.shape[0], em.shape[1]
    n_chunks = s_len // CHUNK
    half = n_chunks // 2
    tags_i = np.asarray(tags).reshape(b_all, s_len).astype(np.int64)

    # denominator emissions: em + start @ s=0, + end @ s=S-1
    em_d = em.copy()
    em_d[:, 0, :] += start
    em_d[:, -1, :] += end
    em_b = em_d.astype(BF16_NP)
    # em_x[c, row, j, b]: rows 0:64 fwd t of chunk c (s = 128c + j);
    # rows 64:128 bwd t of chunk n_chunks-1-c with j reversed
    fwd = em_b[:, :half * 128, :].reshape(b_all, half, 128, T)
    fwd = fwd.transpose(1, 3, 2, 0)                    # [c, t, j, b]
    bwd = em_b[:, half * 128:, :].reshape(b_all, half, 128, T)
    bwd = bwd[:, ::-1, ::-1, :].transpose(1, 3, 2, 0)  # [c, t, j, b]
    em_x = np.concatenate([fwd, bwd], axis=1)          # [c, 128, 128, b]

    # numerator emissions: em + start @ s=0 + end @ s=S-1
    #                      + trans[tag_{s-1}, :] @ s>=1
    em_n = em_d
    em_n[:, 1:, :] += trans[tags_i[:, :-1]]
    em_nb = em_n.astype(BF16_NP)
    # em_n[g, s, b, t] (natural order per chunk)
    em_nb = em_nb.reshape(b_all, n_chunks, 128, T).transpose(1, 2, 0, 3)

    # scatter indices: nidx[g, p, i] = (i mod hb)*T + tag[b0+i, 128g+p]
    # where hb = per-core half-lane count (each local_scatter covers hb lanes)
    hb = max(b_all // ncores // 2, 1)
    tg = tags_i.reshape(b_all, n_chunks, 128).transpose(1, 2, 0)  # [g, p, b]
    lane_base = (np.arange(b_all) % hb) * T                       # [b]
    nidx_all = (tg + lane_base[None, None, :]).astype(np.int16)   # [g, p, b]
    ident, bones, bsel = _consts()
    bsh = b_all // ncores
    in_maps = []
    for cidx in range(ncores):
        sl = slice(cidx * bsh, (cidx + 1) * bsh)
        in_maps.append({
            "emx": np.ascontiguousarray(em_x[:, :, :, sl]),
            "emn": np.ascontiguousarray(em_nb[:, :, sl, :]),
            "nidx": np.ascontiguousarray(nidx_all[:, :, sl]),
            "trans": trans,
            "bones": bones,
            "bsel": bsel,
            "ident": ident,
        })
    return in_maps


def kernel(emissions, start_transitions, end_transitions, transitions,
           tags, mask):
    """Full-input entry point; shards over 8 NeuronCores internally."""
    from concourse.bass_utils import run_bass_kernel_spmd

    emissions = np.asarray(emissions)
    assert emissions.shape == (B, S, T)
    assert (np.asarray(mask) != 0).all(), "kernel assumes all-ones mask"

    in_maps = make_in_maps(emissions, start_transitions, end_transitions,
                           transitions, tags)
    nc = _get_nc()
    res = run_bass_kernel_spmd(nc, in_maps, core_ids=list(range(NCORES)))

    n_rounds = S // 2 - 1
    nren = (n_rounds - 1) // RN
    num_total = 0.0
    den_total = 0.0
    for cidx in range(NCORES):
        r = res.results[cidx]
        num_total += float(r["gsum"][0, 0])
        z = r["zraw"].astype(np.float64).reshape(BSH)
        rml = r["rml"].astype(np.float64).reshape(2, nren, BSH)
        den = np.log(z) - np.log(rml).sum(axis=(0, 1)) + S * C0
        den_total += float(den.sum())
    loss = (den_total - num_total) / float(B)
    return np.float32(loss)


# revision 3
# speedup vs baseline: 2.3067x; 2.3067x over previous
"""CRF negative-log-likelihood loss kernel for Trainium2 (8 NeuronCores).

Problem: B=256, S=2048, T=64 CRF loss (torchcrf-style), mask all-ones.

Strategy
--------
Data-parallel over batch: each of the 8 cores gets 32 batch rows.

Denominator (log-partition): forward/backward meet-in-the-middle in the
exp domain.  The two chains are packed into one 128-partition stream
(block-diagonal lhsT: W for W^T @ E on top, W^T for W @ C below) so
each of the 1023 rounds is one PE matmul [128,128]@[128,32] plus one
DVE multiply [128,32] -- the DVE multiply (PSUM-read, 158 ns) is the
pacing cost.  A constant per-step prescale c0 keeps fp32 in range;
every RN rounds the state is renormalized by the bf16 reciprocal of
its per-direction mass.  The raw bf16 reciprocals (exactly as applied)
and the raw stitched partition value z are shipped to the host, which
reconstructs  den = ln(z) - sum ln(rmass) + S*c0.  No device-side Ln
-> the ACT engine keeps its Exp table loaded all kernel (no table
swaps).

Numerator: host folds start/end transitions AND the gold-path
transition rows (trans[tag_{s-1}, :]) into a separate numerator
emissions tensor em_n, so the numerator reduces to
sum_{b,s} em_n[b, s, tag[b,s]].  On device: per chunk, GpSimd
local_scatter builds the one-hot selection mask from host-prepared
int16 indices (2 instructions / chunk), then the PE contracts
one-hot^T @ em_n with two lanes packed per 128-wide stationary
(16 matmuls / chunk) accumulating into a single persistent PSUM tile;
the numerator is trace(accumulator), extracted once at the end.  The
DVE is left with only the recurrence.

Emissions travel as bf16 (half the DMA bytes); exp() output X stays
f32, stored j-major ([128, j, b]) so the per-round DVE read is
contiguous.

Per-core outputs: zraw[1,32] f32, rml[2, nren*32] bf16, gsum[1,2] f32.
Host: den_b = ln z_b - sum_r ln rml[:, r, b] + S*c0;
loss = (den_tot - num_tot) / B.
"""

import contextlib

import numpy as np
import ml_dtypes

F32_NP = np.float32
BF16_NP = ml_dtypes.bfloat16

B, S, T = 256, 2048, 64
NCORES = 8
BSH = B // NCORES  # 32
CHUNK = 128
C0 = 4.8204  # ~ ln(64 * e^0.5 * sinh(1)) : expected per-step log growth
RN = 128  # renorm every RN rounds (validated vs f64: rel err 1.1e-5, state range e-8..17)

_NC_CACHE = {}


def build(n_chunks=16, bsh=BSH, nrep=1, fake_x=False, no_num=False,
          no_rounds=False, rn=RN, pround_bufs=4, spool_bufs=6,
          fake_x_dma=False, num_mode="pe2", act_k=0, x_bf16=False):
    """Build + compile the per-core Bass module. n_chunks*128 = seq len."""
    import concourse.bacc as bacc
    import concourse.mybir as mybir
    import concourse.tile as tile

    F32 = mybir.dt.float32
    BF16 = mybir.dt.bfloat16
    AF = mybir.ActivationFunctionType
    ALU = mybir.AluOpType

    s_len = n_chunks * CHUNK
    half = n_chunks // 2
    assert half * 2 == n_chunks and half >= 1
    n_rounds = half * CHUNK - 1
    nren = max((n_rounds - 1) // rn, 0) if not no_rounds else 0
    n_windows = half  # one window per 128 rounds

    nc = bacc.Bacc("TRN2", target_bir_lowering=False, debug=False,
                   num_devices=NCORES)

    em_x_d = nc.dram_tensor("emx", [half, 128, 128, bsh], BF16,
                            kind="ExternalInput")
    em_n_d = nc.dram_tensor("emn", [n_chunks, 128, bsh, T], BF16,
                            kind="ExternalInput")
    nidx_d = nc.dram_tensor("nidx", [n_chunks, 128, bsh], mybir.dt.int16,
                            kind="ExternalInput")
    trans_d = nc.dram_tensor("trans", [T, T], F32, kind="ExternalInput")
    bones_d = nc.dram_tensor("bones", [128, 2], BF16, kind="ExternalInput")
    bsel_d = nc.dram_tensor("bsel", [2, 128], BF16, kind="ExternalInput")
    ident_d = nc.dram_tensor("ident", [128, 128], F32, kind="ExternalInput")
    z_d = nc.dram_tensor("zraw", [1, bsh], F32, kind="ExternalOutput")
    rml_d = nc.dram_tensor("rml", [2, max(nren, 1) * bsh], BF16,
                           kind="ExternalOutput")
    gsum_d = nc.dram_tensor("gsum", [1, 2], F32, kind="ExternalOutput")

    with tile.TileContext(nc) as tc, nc.allow_low_precision(
            reason="bf16 state/weights validated against f64 reference"):
        with (
            tc.tile_pool(name="consts", bufs=1) as consts,
            tc.tile_pool(name="xchunk", bufs=3) as xpool,
            tc.tile_pool(name="xraw", bufs=3) as xrawpool,
            tc.tile_pool(name="emt", bufs=4) as empool,
            tc.tile_pool(name="oht", bufs=3) as ohpool,
            tc.tile_pool(name="state", bufs=spool_bufs) as spool,
            tc.tile_pool(name="small", bufs=4) as smallpool,
            tc.tile_pool(name="prod", bufs=3) as prodpool,
            tc.tile_pool(name="pround", bufs=pround_bufs,
                         space="PSUM") as pround,
            tc.tile_pool(name="pmisc", bufs=1, space="PSUM") as pmisc,
            tc.tile_pool(name="pnum", bufs=1, space="PSUM") as pnum,
        ):
            rep_ctx = (tc.For_i(0, nrep, 1) if nrep > 1
                       else contextlib.nullcontext())
            with rep_ctx:
                # ---------------- constants / setup ----------------
                ident = consts.tile([128, 128], F32, tag="ident")
                nc.sync.dma_start(ident[:], ident_d.ap())
                trans_sb = consts.tile([T, T], F32, tag="trans")
                nc.sync.dma_start(trans_sb[:], trans_d.ap())

                # block-diagonal lhsT (bf16): top-left W (for W^T @ E),
                # bottom-right W^T (for W @ C)
                blockw = consts.tile([128, 128], BF16, tag="blockw")
                nc.vector.memset(blockw[:], 0.0)
                nc.scalar.activation(blockw[0:T, 0:T], trans_sb[:], AF.Exp)
                tp = pmisc.tile([128, 128], F32, tag="m128")
                nc.tensor.matmul(tp[0:T, 0:T], trans_sb[:], ident[0:T, 0:T],
                                 start=True, stop=True)
                nc.scalar.activation(blockw[T:128, T:128], tp[0:T, 0:T],
                                     AF.Exp)

                blockones = consts.tile([128, 2], BF16, tag="blockones")
                nc.sync.dma_start(blockones[:], bones_d.ap())
                blocksel = consts.tile([2, 128], BF16, tag="blocksel")
                nc.sync.dma_start(blocksel[:], bsel_d.ap())
                ones64 = consts.tile([T, 1], F32, tag="ones64")
                nc.vector.memset(ones64[:], 1.0)
                ones128 = consts.tile([128, 1], F32, tag="ones128")
                nc.vector.memset(ones128[:], 1.0)
                ones128b = consts.tile([128, 1], BF16, tag="ones128b")
                nc.vector.memset(ones128b[:], 1.0)
                negc0 = consts.tile([128, 1], F32, tag="negc0")
                nc.vector.memset(negc0[:], -C0)

                # renorm log buffer: applied bf16 reciprocals
                rml = consts.tile([2, max(nren, 1) * bsh], BF16, tag="rml")
                if nren == 0:
                    nc.vector.memset(rml[:], 1.0)

                # scatter indices: nidx[p, g, i] = host-prepared int16 offsets
                if not no_num:
                    nidx = consts.tile([128, n_chunks, bsh], mybir.dt.int16,
                                       tag="nidx")
                    nc.sync.dma_start(
                        nidx[:], nidx_d.ap().rearrange("g p i -> p g i"))
                    ones_b = consts.tile([128, bsh], BF16, tag="onesb")
                    nc.vector.memset(ones_b[:], 1.0)

                # numerator accumulator
                nsum = 4  # PE column-sum slices per chunk
                wsum = (bsh * T) // nsum
                if not no_num:
                    if num_mode == "pe2":
                        num_ps = pnum.tile([128, 128], F32, tag="num_ps")
                        num_state = {"first": True, "last": None}
                    elif num_mode == "pool":
                        num_ps = pnum.tile([1, wsum], F32, tag="num_ps")
                        num_state = {"first": True, "last": None}
                    elif num_mode == "ttr":
                        acc = consts.tile([128, n_chunks], F32, tag="acc")
                        gscr = consts.tile([128, bsh, T], BF16, tag="gscr")
                    else:  # stt
                        nsub = 4
                        bsub = bsh // nsub
                        acc = consts.tile([128, n_chunks * nsub], F32,
                                          tag="acc")
                        gscr = consts.tile([128, bsub, T], BF16, tag="gscr")

                emg = {}      # em_n chunk g -> tile [128, bsh, T] bf16
                ohg = {}      # one-hot chunk g -> tile [128, bsh, T] bf16
                prg = {}      # product chunk g -> tile [128, bsh, T] bf16
                exraw = {}    # x-chunk c -> tile [128, 128, bsh] bf16

                def dma_x(c):
                    xr = xrawpool.tile([128, 128, bsh], BF16, tag="xr")
                    nc.sync.dma_start(xr[:], em_x_d.ap()[c])
                    exraw[c] = xr

                def dma_emn(g):
                    eg = empool.tile([128, bsh, T], BF16, tag="em")
                    nc.sync.dma_start(eg[:], em_n_d.ap()[g])
                    emg[g] = eg

                def num_quanta(g):
                    """Chunk-g numerator: Pool one-hot scatter + DVE accum."""
                    qs = []
                    if no_num:
                        return qs
                    hb = bsh // 2

                    def q_oh(g=g, h=0):
                        if h == 0:
                            oh = ohpool.tile([128, bsh, T], BF16, tag="oh")
                            ohg[g] = oh
                        nc.gpsimd.local_scatter(
                            ohg[g][:, h * hb:(h + 1) * hb, :],
                            ones_b[:, h * hb:(h + 1) * hb],
                            nidx[:, g, h * hb:(h + 1) * hb],
                            channels=128, num_elems=hb * T, num_idxs=hb)
                    qs.append(lambda g=g: q_oh(g, 0))
                    qs.append(lambda g=g: q_oh(g, 1))
                    if num_mode == "pe2":
                        npair = bsh // 2
                        for i in range(npair):
                            def q(g=g, i=i, last=(i == npair - 1)):
                                mm = nc.tensor.matmul(
                                    num_ps[:], ohg[g][:, 2 * i:2 * i + 2, :],
                                    emg[g][:, 2 * i:2 * i + 2, :],
                                    start=num_state["first"], stop=False,
                                    skip_group_check=True)
                                num_state["first"] = False
                                num_state["last"] = mm
                                if last:
                                    del emg[g]
                                    del ohg[g]
                            qs.append(q)
                    elif num_mode == "pool":
                        def q_mul(g=g, h=0):
                            if h == 0:
                                pr = prodpool.tile([128, bsh, T], BF16,
                                                   tag="pr")
                                prg[g] = pr
                            sl = slice(h * hb, (h + 1) * hb)
                            nc.gpsimd.tensor_mul(
                                prg[g][:, sl, :], emg[g][:, sl, :],
                                ohg[g][:, sl, :])
                            if h == 1:
                                del emg[g]
                                del ohg[g]
                        qs.append(lambda g=g: q_mul(g, 0))
                        qs.append(lambda g=g: q_mul(g, 1))
                        bsl = bsh // nsum
                        for si in range(nsum):
                            def q(g=g, si=si, last=(si == nsum - 1)):
                                mm = nc.tensor.matmul(
                                    num_ps[:], ones128b[:],
                                    prg[g][:, si * bsl:(si + 1) * bsl, :],
                                    start=num_state["first"], stop=False,
                                    skip_group_check=True)
                                num_state["first"] = False
                                num_state["last"] = mm
                                if last:
                                    del prg[g]
                            qs.append(q)
                    elif num_mode == "ttr":
                        def q_ttr(g=g):
                            nc.vector.tensor_tensor_reduce(
                                gscr[:], emg[g][:], ohg[g][:],
                                1.0, 0.0, op0=ALU.mult, op1=ALU.add,
                                accum_out=acc[:, g:g + 1])
                            del emg[g]
                            del ohg[g]
                        qs.append(q_ttr)
                    else:  # stt
                        for si in range(nsub):
                            def q(g=g, si=si, last=(si == nsub - 1)):
                                sl = slice(si * bsub, (si + 1) * bsub)
                                col = g * nsub + si
                                nc.vector.scalar_tensor_tensor(
                                    gscr[:], emg[g][:, sl, :], 1.0,
                                    ohg[g][:, sl, :],
                                    op0=ALU.bypass, op1=ALU.mult,
                                    accum_out=acc[:, col:col + 1])
                                if last:
                                    del emg[g]
                                    del ohg[g]
                            qs.append(q)
                    return qs

                XDT = BF16 if x_bf16 else F32

                def x_quanta(c):
                    """ACT-exp quanta producing X chunk c from em_x."""
                    xc = xpool.tile([128, 128, bsh], XDT, tag="xc")
                    if fake_x or fake_x_dma:
                        def q():
                            nc.vector.memset(xc[:], 0.0133)
                        return xc, [q]
                    qs = []
                    for hj in range(4):
                        def q(hj=hj):
                            sl = slice(hj * 32, (hj + 1) * 32)
                            nc.scalar.activation(
                                xc[:, sl, :], exraw[c][:, sl, :],
                                AF.Exp, bias=negc0[:])
                            if hj == 3:
                                del exraw[c]
                        qs.append(q)
                    return xc, qs

                # ---------------- main pipeline ----------------
                from collections import deque
                bg = deque()
                xchunks = {}
                if fake_x:
                    for c in (0, 1):
                        if c <= half - 1:
                            xc, qs = x_quanta(c)
                            [q() for q in qs]
                            xchunks[c] = xc
                else:
                    dma_x(0)
                    if half > 1:
                        dma_x(1)
                    if half > 2:
                        dma_x(2)
                    if not no_num:
                        dma_emn(0)
                        dma_emn(1)
                    xc, qs = x_quanta(0)
                    [q() for q in qs]
                    xchunks[0] = xc
                    if half > 1:
                        xc, qs = x_quanta(1)
                        [q() for q in qs]
                        xchunks[1] = xc

                state = spool.tile([128, bsh], BF16, tag="st")
                nc.vector.tensor_copy(state[:], xchunks[0][:, 0, :])

                ren_i = 0
                r_end = 0 if no_rounds else n_rounds
                for r in range(1, r_end + 1):
                    c, j = r >> 7, r & 127
                    if j == 1:
                        if not fake_x and c + 3 <= half - 1:
                            dma_x(c + 3)
                        if not fake_x and not no_num:
                            for g in (2 * (c + 1), 2 * (c + 1) + 1):
                                if g <= n_chunks - 1:
                                    dma_emn(g)
                        if c + 2 <= half - 1:
                            xc, qs = x_quanta(c + 2)
                            xchunks[c + 2] = xc
                            bg.extend(qs)
                            xchunks.pop(c - 1, None)
                        if not fake_x:
                            bg.extend(num_quanta(2 * c))
                            bg.extend(num_quanta(2 * c + 1))
                    if bg:
                        bg.popleft()()
                    p = pround.tile([128, bsh], F32, tag="p")
                    nc.tensor.matmul(p[:], blockw[:], state[:],
                                     start=True, stop=True)
                    state = spool.tile([128, bsh], BF16, tag="st")
                    if (r % 16) < act_k:
                        psb = smallpool.tile([128, bsh], BF16, tag="psb")
                        nc.scalar.activation(psb[:], p[:], AF.Copy)
                        nc.vector.tensor_mul(state[:], psb[:],
                                             xchunks[c][:, j, :])
                    else:
                        nc.vector.tensor_mul(state[:], p[:],
                                             xchunks[c][:, j, :])

                    if r % rn == 0 and r < n_rounds and ren_i < nren:
                        sl = slice(ren_i * bsh, (ren_i + 1) * bsh)
                        mass = pmisc.tile([2, bsh], F32, tag="m2")
                        nc.tensor.matmul(mass[:], blockones[:], state[:],
                                         start=True, stop=True)
                        nc.vector.reciprocal(rml[:, sl], mass[:])
                        rbc = pmisc.tile([128, 128], F32, tag="m128")
                        nc.tensor.matmul(rbc[:, 0:bsh], blocksel[:],
                                         rml[:, sl], start=True, stop=True)
                        nstate = spool.tile([128, bsh], BF16, tag="st")
                        nc.vector.tensor_mul(nstate[:], state[:],
                                             rbc[:, 0:bsh])
                        state = nstate
                        ren_i += 1

                while bg:
                    bg.popleft()()
                if no_rounds and not fake_x and not no_num:
                    for g in range(n_chunks):
                        if g >= 2:
                            dma_emn(g)
                        for q in num_quanta(g):
                            q()

                # ---------------- final combine ----------------
                # beta = W @ C on partitions 0..63 (aligned base-64 matmul)
                pf = pround.tile([128, bsh], F32, tag="p")
                nc.tensor.matmul(pf[0:T, :], blockw[T:128, T:128],
                                 state[T:128, :], start=True, stop=True)
                y = smallpool.tile([T, bsh], F32, tag="y")
                nc.vector.tensor_mul(y[:], state[0:T, :], pf[0:T, :])
                z = pmisc.tile([2, bsh], F32, tag="m2")
                nc.tensor.matmul(z[0:1, :], ones64[:], y[:],
                                 start=True, stop=True)
                z_sb = smallpool.tile([1, bsh], F32, tag="zsb")
                nc.vector.tensor_copy(z_sb[:], z[0:1, :])
                nc.sync.dma_start(z_d.ap(), z_sb[:])
                nc.sync.dma_start(rml_d.ap(), rml[:])

                # numerator finish
                gsb = smallpool.tile([1, 2], F32, tag="gsb")
                nc.vector.memset(gsb[:], 0.0)
                if not no_num:
                    if num_mode == "pe2":
                        if num_state["last"] is not None:
                            num_state["last"].ins.stop_tensor_calc = True
                            tr1 = smallpool.tile([128, 1], F32, tag="tr1")
                            trscr = smallpool.tile([128, 128], F32,
                                                   tag="trscr")
                            nc.vector.scalar_tensor_tensor(
                                trscr[:], ident[:], 1.0, num_ps[:],
                                op0=ALU.bypass, op1=ALU.mult,
                                accum_out=tr1[:])
                            gps = pmisc.tile([2, bsh], F32, tag="m2")
                            nc.tensor.matmul(gps[0:1, 0:1], ones128[:],
                                             tr1[:], start=True, stop=True)
                            nc.vector.tensor_copy(gsb[:, 0:1],
                                                  gps[0:1, 0:1])
                    elif num_mode == "pool":
                        if num_state["last"] is not None:
                            num_state["last"].ins.stop_tensor_calc = True
                            nc.vector.tensor_reduce(
                                gsb[:, 0:1], num_ps[:],
                                mybir.AxisListType.X, ALU.add)
                    else:
                        acc1 = smallpool.tile([128, 1], F32, tag="acc1")
                        nc.vector.tensor_reduce(
                            acc1[:], acc[:], mybir.AxisListType.X, ALU.add)
                        gps = pmisc.tile([2, bsh], F32, tag="m2")
                        nc.tensor.matmul(gps[0:1, 0:1], ones128[:], acc1[:],
                                         start=True, stop=True)
                        nc.vector.tensor_copy(gsb[:, 0:1], gps[0:1, 0:1])
                nc.sync.dma_start(gsum_d.ap(), gsb[:])

    nc.compile()
    return nc


def _get_nc(n_chunks=16, bsh=BSH):
    key = (n_chunks, bsh)
    if key not in _NC_CACHE:
        _NC_CACHE[key] = build(n_chunks, bsh)
    return _NC_CACHE[key]


def _consts():
    ident = np.eye(128, dtype=F32_NP)
    bones = np.zeros((128, 2), dtype=F32_NP)
    bones[0:T, 0] = 1.0
    bones[T:128, 1] = 1.0
    bsel = np.zeros((2, 128), dtype=F32_NP)
    bsel[0, 0:T] = 1.0
    bsel[1, T:128] = 1.0
    return ident, bones.astype(BF16_NP), bsel.astype(BF16_NP)


def make_in_maps(emissions, start_transitions, end_transitions, transitions,
                 tags, ncores=NCORES):
    """Host prep: fold start/end into em (denominator path), fold
    start/end + gold-path transition rows into em_n (numerator path),
    convert to bf16, build the DMA-friendly layouts, shard over cores."""
    em = np.asarray(emissions, dtype=F32_NP)
    start = np.asarray(start_transitions, dtype=F32_NP)
    end = np.asarray(end_transitions, dtype=F32_NP)
    trans = np.asarray(transitions, dtype=F32_NP).reshape(T, T)
    b_all, s_len = em.shape[0], em.shape[1]
    n_chunks = s_len // CHUNK
    half = n_chunks // 2
    tags_i = np.asarray(tags).reshape(b_all, s_len).astype(np.int64)

    # denominator emissions: em + start @ s=0, + end @ s=S-1
    em_d = em.copy()
    em_d[:, 0, :] += start
    em_d[:, -1, :] += end
    em_b = em_d.astype(BF16_NP)
    # em_x[c, row, j, b]: rows 0:64 fwd t of chunk c (s = 128c + j);
    # rows 64:128 bwd t of chunk n_chunks-1-c with j reversed
    fwd = em_b[:, :half * 128, :].reshape(b_all, half, 128, T)
    fwd = fwd.transpose(1, 3, 2, 0)                    # [c, t, j, b]
    bwd = em_b[:, half * 128:, :].reshape(b_all, half, 128, T)
    bwd = bwd[:, ::-1, ::-1, :].transpose(1, 3, 2, 0)  # [c, t, j, b]
    em_x = np.concatenate([fwd, bwd], axis=1)          # [c, 128, 128, b]

    # numerator emissions: em + start @ s=0 + end @ s=S-1
    #                      + trans[tag_{s-1}, :] @ s>=1
    em_n = em_d
    em_n[:, 1:, :] += trans[tags_i[:, :-1]]
    em_nb = em_n.astype(BF16_NP)
    # em_n[g, s, b, t] (natural order per chunk)
    em_nb = em_nb.reshape(b_all, n_chunks, 128, T).transpose(1, 2, 0, 3)

    # scatter indices: nidx[g, p, i] = (i mod hb)*T + tag[b0+i, 128g+p]
    # where hb = per-core half-lane count (each local_scatter covers hb lanes)
    hb = max(b_all // ncores // 2, 1)
    tg = tags_i.reshape(b_all, n_chunks, 128).transpose(1, 2, 0)  # [g, p, b]
    lane_base = (np.arange(b_all) % hb) * T                       # [b]
    nidx_all = (tg + lane_base[None, None, :]).astype(np.int16)   # [g, p, b]
    ident, bones, bsel = _consts()
    bsh = b_all // ncores
    in_maps = []
    for cidx in range(ncores):
        sl = slice(cidx * bsh, (cidx + 1) * bsh)
        in_maps.append({
            "emx": np.ascontiguousarray(em_x[:, :, :, sl]),
            "emn": np.ascontiguousarray(em_nb[:, :, sl, :]),
            "nidx": np.ascontiguousarray(nidx_all[:, :, sl]),
            "trans": trans,
            "bones": bones,
            "bsel": bsel,
            "ident": ident,
        })
    return in_maps


def kernel(emissions, start_transitions, end_transitions, transitions,
           tags, mask):
    """Full-input entry point; shards over 8 NeuronCores internally."""
    from concourse.bass_utils import run_bass_kernel_spmd

    emissions = np.asarray(emissions)
    assert emissions.shape == (B, S, T)
    assert (np.asarray(mask) != 0).all(), "kernel assumes all-ones mask"

    in_maps = make_in_maps(emissions, start_transitions, end_transitions,
                           transitions, tags)
    nc = _get_nc()
    res = run_bass_kernel_spmd(nc, in_maps, core_ids=list(range(NCORES)))

    n_rounds = S // 2 - 1
    nren = (n_rounds - 1) // RN
    num_total = 0.0
    den_total = 0.0
    for cidx in range(NCORES):
        r = res.results[cidx]
        num_total += float(r["gsum"][0, 0])
        z = r["zraw"].astype(np.float64).reshape(BSH)
        rml = r["rml"].astype(np.float64).reshape(2, nren, BSH)
        den = np.log(z) - np.log(rml).sum(axis=(0, 1)) + S * C0
        den_total += float(den.sum())
    loss = (den_total - num_total) / float(B)
    return np.float32(loss)
